# revision 6
# baseline (speedup 1.0000x reference)
"""Batch-hard triplet loss on 8 Trainium2 NeuronCores.

Math (matches the reference exactly up to fp rounding):
  d_ij   = ||h_i||^2 + ||h_j||^2 - 2 h_i.h_j, clamped to [EPS, inf)
  hp_i   = max over j (same label, j != i) of d_ij
  hn_i   = 2nd-smallest over j (different label) of d_ij
  loss_i = max(hp_i - hn_i + ALPHA, 0)
  out    = sum(loss_i[loss_i > EPS]) / count(loss_i > EPS)

Device strategy: rows are sharded over 8 cores (1024 each). Each core mines
from the quantity

  p_ij = 2 h_i.h_j - ||h_j||^2 - BIG * [label_i == label_j]

Row-constant terms (||h_i||^2, the EPS clamp) cancel in hp - hn, so they are
never computed.  With t_ij := d_ij - ||h_i||^2 = -p_ij - BIG*eq:
  hp_i = -min_j(p_ij) - BIG        (positives carry -BIG, dominate the min;
                                    Sterbenz: the BIG subtraction is exact)
  hn_i = -max8(p_i)[1]             (negatives are the largest p; the DVE Max8
                                    instruction gives the top-8 descending, so
                                    element 1 is the 2nd-smallest distance,
                                    with tie multiplicity matching top_k)
  loss_i = max( max8[1] - min + (ALPHA - BIG), 0 )   (clamp applied on host)

Rows are sorted by label and each core's B columns are rotated so every
own-class (positive) column sits in the first hpj = 3 j-blocks.  Only those
blocks need the -BIG mask, which rides a 5th e5m2 DoubleRow matmul (also
carrying ||h_j||^2 as a 6-term e5m2 expansion).  The other 13 j-blocks get
2 h_i.h_j from 4 e4m3 DoubleRow matmuls (K=256 each); their -||h_j||^2 is
added off the PE in exact f32: the Scalar (Act) engine drains each PSUM tile
into a [128, 2*JB] SBUF staging pair (GpSimd cannot touch PSUM on TRN2),
GpSimd adds the resident negated-norm row, and the DVE Max8 then runs once
per staged PAIR of j-blocks ([128, 1024] per op), which both removes the 5th
matmul from 13/16 tiles and cuts the DVE instruction count roughly in half.

Per-row losses leave the device as a [128, m_chunks] tile (partition p, row
chunk m) in one contiguous DMA; the host transposes and does the masked mean.
"""

import functools

import numpy as np
import ml_dtypes

import concourse.bacc as bacc
import concourse.tile as tile
from concourse import mybir
from concourse.bass_utils import run_bass_kernel_spmd

FP8E4 = mybir.dt.float8e4
FP8E5 = mybir.dt.float8e5
F32 = mybir.dt.float32
BF16 = mybir.dt.bfloat16
E4 = ml_dtypes.float8_e4m3
E5 = ml_dtypes.float8_e5m2

N, D, C = 8192, 1024, 128
NCORES = 8
P = 128
JB = 512  # matmul moving free dim = one fp32 PSUM bank
HPJ = 3   # j-blocks that can contain positive (own-class) columns
ALPHA = 0.1
EPS = 1e-7
BIG = 8192.0
NNORM = 6  # e5m2 expansion terms for ||h_j||^2 in the masked blocks
MODE = "fp8"


def build_program(rows, n, d, c, jb, mode=MODE, psum_bufs=8, b_bufs=4):
    """Emit the per-core Bass/Tile program (identical on all cores)."""
    kh = d // P
    m_chunks = rows // P
    nj = n // jb
    hpj = min(nj, HPJ)
    assert rows % P == 0 and d % P == 0 and n % jb == 0 and c <= P
    assert kh % 2 == 0

    nc = bacc.Bacc("TRN2", target_bir_lowering=False)
    A4 = nc.dram_tensor("A4", [P, kh, rows], FP8E4, kind="ExternalInput")
    B4 = nc.dram_tensor("B4", [P, kh, n], FP8E4, kind="ExternalInput")
    A5 = nc.dram_tensor("A5", [P, 2, rows], FP8E5, kind="ExternalInput")
    B5 = nc.dram_tensor("B5", [P, 2, hpj * jb], FP8E5, kind="ExternalInput")
    NB = nc.dram_tensor("NB", [P, (nj - hpj) * jb], F32, kind="ExternalInput")
    loss = nc.dram_tensor("loss", [P, m_chunks], F32, kind="ExternalOutput")

    assert nj % 2 == 0

    with tile.TileContext(nc) as tc:
        with (
            tc.tile_pool(name="apool", bufs=1) as apool,
            tc.tile_pool(name="bpool", bufs=b_bufs) as bpool,
            tc.tile_pool(name="psum", bufs=psum_bufs, space="PSUM") as pp,
            tc.tile_pool(name="mpool", bufs=1) as mpool,
            tc.tile_pool(name="spool", bufs=2) as spool,
            tc.tile_pool(name="fpool", bufs=6) as fpool,
        ):
            # Warm the PE HAM clock gate while the first DMAs land: dummy
            # matmuls on a zeroed tile keep the PE busy through its
            # 4096-cycle activity window so real matmuls run at 2.4 GHz.
            wsrc = apool.tile([1, 16 + jb], BF16, tag="wsrc")
            nc.vector.memset(wsrc[:], 0.0)
            wps = pp.tile([P, jb], F32, name="ps", tag="ps")
            for _ in range(8):
                nc.tensor.matmul(wps[:16, :], wsrc[:1, :16], wsrc[:1, 16:],
                                 start=True, stop=True)

            def load_b(j):
                js = slice(j * jb, (j + 1) * jb)
                b4 = bpool.tile([P, kh, jb], FP8E4, tag="b4", name="b4")
                nc.sync.dma_start(out=b4[:], in_=B4[:, :, js])
                if j >= hpj:
                    return (b4, None)
                b5 = bpool.tile([P, 2, jb], FP8E5, tag="b5", name="b5")
                nc.sync.dma_start(out=b5[:], in_=B5[:, :, js])
                return (b4, b5)

            # First moving block before the stationary A so compute can
            # start as soon as possible.
            b_pre = load_b(0)

            # Stationary A, resident all kernel, loaded as one DMA per row
            # chunk so chunk m's operands land just before the PE needs
            # them (one big transfer stalled the pipeline and re-tripped
            # the HAM throttle).
            a4m = []
            a5 = None
            b_pre2 = None
            for m in range(m_chunks):
                ms = slice(m * P, (m + 1) * P)
                t = apool.tile([P, kh, P], FP8E4, tag=f"a4m{m}",
                               name=f"a4m{m}")
                nc.sync.dma_start(out=t[:], in_=A4[:, :, ms])
                a4m.append(t)
                if m == 0:
                    a5 = apool.tile([P, 2, rows], FP8E5, tag="a5", name="a5")
                    nc.sync.dma_start(out=a5[:], in_=A5[:])
                if m == 2 and nj > 1:
                    # Prefetch the second moving block ahead of the
                    # remaining stationary chunks so j=1 never stalls.
                    b_pre2 = load_b(1)
            # Negated column norms for the unmasked blocks, resident f32.
            normb = apool.tile([P, (nj - hpj) * jb], F32, tag="normb")
            nc.sync.dma_start(out=normb[:], in_=NB[:])

            # Per-row-chunk partial mining results, merged after the j loop.
            # Max8 runs once per staged pair of j-blocks -> nj/2 slots of 8.
            v8 = [mpool.tile([P, nj * 4], F32, tag=f"v8_{m}", name=f"v8_{m}")
                  for m in range(m_chunks)]
            gmin = [mpool.tile([P, hpj], F32, tag=f"gm_{m}", name=f"gmin_{m}")
                    for m in range(m_chunks)]

            stage8 = mpool.tile([P, m_chunks], F32, tag="stage8")
            stages = [None] * m_chunks

            for j in range(nj):
                if j == 0:
                    b4, b5 = b_pre
                elif j == 1 and b_pre2 is not None:
                    b4, b5 = b_pre2
                else:
                    b4, b5 = load_b(j)

                for m in range(m_chunks):
                    ps = pp.tile([P, jb], F32, name="ps", tag="ps")
                    at = a4m[m]
                    for t in range(kh // 2):
                        nc.tensor.matmul(
                            ps[:], at[:, 2 * t:2 * t + 2, :],
                            b4[:, 2 * t:2 * t + 2, :],
                            start=(t == 0),
                            stop=(t == kh // 2 - 1 and j >= hpj),
                            perf_mode=mybir.MatmulPerfMode.DoubleRow)
                    if j < hpj:
                        # Masked block: -BIG one-hot + e5m2 norm expansion.
                        nc.tensor.matmul(
                            ps[:], a5[:, :, m * P:(m + 1) * P], b5[:],
                            start=False, stop=True,
                            perf_mode=mybir.MatmulPerfMode.DoubleRow)
                        # hp mining straight off PSUM, in parallel with the
                        # Act-engine drain below.
                        nc.vector.tensor_reduce(gmin[m][:, j:j + 1], ps[:],
                                                axis=mybir.AxisListType.X,
                                                op=mybir.AluOpType.min)

                    if j % 2 == 0:
                        stages[m] = spool.tile([P, 2 * jb], F32,
                                               tag=f"st{m}", name=f"st{m}")
                    st = stages[m]
                    half = st[:, (j % 2) * jb:(j % 2 + 1) * jb]
                    nc.scalar.copy(half, ps[:])
                    if j >= hpj:
                        # Unmasked block: exact f32 -||h_j||^2 off the PE.
                        jo = (j - hpj) * jb
                        nc.gpsimd.tensor_add(half, half, normb[:, jo:jo + jb])
                    if j % 2 == 1:
                        nc.vector.max(v8[m][:, (j // 2) * 8:(j // 2 + 1) * 8],
                                      st[:])

                    if j == nj - 1:
                        # Final merge for this row chunk, interleaved so it
                        # overlaps the remaining row chunks' matmuls.
                        vf = fpool.tile([P, 8], F32, tag="vf", name="vf")
                        nc.vector.max(vf[:], v8[m][:])
                        gm = fpool.tile([P, 1], F32, tag="gm", name="gm")
                        nc.vector.tensor_reduce(gm[:], gmin[m][:],
                                                axis=mybir.AxisListType.X,
                                                op=mybir.AluOpType.min)
                        # loss_pre = (v2 + (ALPHA - BIG)) - gmin
                        nc.vector.scalar_tensor_tensor(
                            out=stage8[:, m:m + 1], in0=vf[:, 1:2],
                            scalar=float(ALPHA - BIG), in1=gm[:],
                            op0=mybir.AluOpType.add,
                            op1=mybir.AluOpType.subtract)

            # One contiguous [P, m_chunks] DMA; the host transposes.
            nc.sync.dma_start(out=loss[:], in_=stage8[:])

    nc.compile()
    return nc


def _split_e5(x, terms):
    """Greedy e5m2 expansion: x ~ sum of `terms` e5m2 rows (f64 in/out)."""
    out = []
    r = x.astype(np.float64).copy()
    for _ in range(terms):
        s = r.astype(E5)
        out.append(s)
        r -= s.astype(np.float64)
    return out


def make_inputs(H, labels, n, d, c, ncores, mode=MODE):
    """Host-side packing of the augmented GEMM operands.

    Rows are sorted by label and sharded contiguously.  Each core's B
    columns are rotated so every column whose label appears among that
    core's rows sits in the leading block (always < HPJ * JB columns), which
    lets the device mine the hardest positive from the first HPJ j-blocks
    only and skip the mask matmul everywhere else.  The final masked mean
    is permutation invariant, so neither the sort nor the rotations need
    undoing.
    """
    H = np.ascontiguousarray(np.asarray(H, dtype=np.float32))
    labels = np.asarray(labels).astype(np.int64).ravel()
    kh = d // P
    rows = n // ncores
    nj = n // JB
    hpj = min(nj, HPJ)

    perm = np.argsort(labels, kind="stable")
    H = H[perm]
    labels = labels[perm]
    col_orders = []
    for cix in range(ncores):
        own = np.zeros(n, dtype=bool)
        own[np.isin(labels, labels[cix * rows:(cix + 1) * rows])] = True
        order = np.concatenate([np.nonzero(own)[0], np.nonzero(~own)[0]])
        assert own.sum() <= min(n, hpj * JB), own.sum()
        col_orders.append(order)

    oh = labels[None, :] == np.arange(c, dtype=np.int64)[:, None]  # [c, n]

    Hr = H.astype(E4)
    Hr32 = Hr.astype(np.float32)
    xn = np.einsum("ij,ij->i", Hr.astype(np.float64), Hr.astype(np.float64))
    xsplit = _split_e5(xn, NNORM)

    B4m = Hr.T.reshape(kh, P, n).transpose(1, 0, 2).copy()  # [P, kh, n] e4m3
    B5m = np.zeros((P, 2, n), dtype=E5)
    B5m[:c, 0, :] = oh.astype(E5)
    for t in range(NNORM):
        B5m[t, 1, :] = xsplit[t]
    nbneg = -xn.astype(np.float32)

    in_maps = []
    for cix in range(ncores):
        sl = slice(cix * rows, (cix + 1) * rows)
        order = col_orders[cix]
        A4m = ((2.0 * Hr32[sl].T).astype(E4)
               .reshape(kh, P, rows).transpose(1, 0, 2).copy())
        A5m = np.zeros((P, 2, rows), dtype=E5)
        A5m[:c, 0, :] = (-BIG * oh[:, sl]).astype(E5)
        A5m[:NNORM, 1, :] = -1.0
        NBm = np.ascontiguousarray(np.broadcast_to(
            nbneg[order][hpj * JB:], (P, n - hpj * JB)))
        in_maps.append({"A4": A4m, "B4": B4m[:, :, order],
                        "A5": A5m, "B5": B5m[:, :, order[:hpj * JB]],
                        "NB": NBm})
    return in_maps


@functools.lru_cache(maxsize=2)
def _get_program(mode=MODE):
    return build_program(N // NCORES, N, D, C, JB, mode=mode)


def _finalize(loss_rows):
    loss_all = np.concatenate(
        [np.asarray(l, dtype=np.float64).T.ravel() for l in loss_rows])
    loss_all = np.maximum(loss_all, 0.0)
    rel = loss_all > EPS
    cnt = int(rel.sum())
    if cnt == 0:
        return np.float32(0.0)
    return np.float32(loss_all[rel].sum() / cnt)


def kernel(H, labels):
    in_maps = make_inputs(H, labels, N, D, C, NCORES)
    res = run_bass_kernel_spmd(_get_program(), in_maps, list(range(NCORES)))
    return _finalize([r["loss"] for r in res.results])


# revision 8
# speedup vs baseline: 1.2750x; 1.2750x over previous
"""Batch-hard triplet loss on 8 Trainium2 NeuronCores.

Math (matches the reference up to fp rounding and a tiny truncation noise):
  d_ij   = ||h_i||^2 + ||h_j||^2 - 2 h_i.h_j, clamped to [EPS, inf)
  hp_i   = max over j (same label, j != i) of d_ij
  hn_i   = 2nd-smallest over j (different label) of d_ij
  loss_i = max(hp_i - hn_i + ALPHA, 0)
  out    = sum(loss_i[loss_i > EPS]) / count(loss_i > EPS)

Device strategy: rows are sharded over 8 cores (1024 each). Each core mines
from the quantity

  p_ij = 2 h_i.h_j - ||h_j||^2 - BIG * [label_i == label_j]

Row-constant terms (||h_i||^2, the EPS clamp) cancel in hp - hn, so they are
never computed.  With t_ij := d_ij - ||h_i||^2 = -p_ij - BIG*eq:
  hp_i = -min_j(p_ij) - BIG        (positives carry -BIG, dominate the min;
                                    Sterbenz: the BIG subtraction is exact)
  hn_i = -max8(p_i)[1]             (negatives are the largest p; the DVE Max8
                                    instruction gives the top-8 descending, so
                                    element 1 is the 2nd-smallest distance,
                                    with tie multiplicity matching top_k)
  loss_i = max( max8[1] - min + (ALPHA - BIG), 0 )   (clamp applied on host)

The key trick: the whole p_ij for an unmasked block comes out of FOUR e4m3
DoubleRow matmuls (K=256 each).  The last 3 of the 1024 contraction slots
carry not data but a 3-term e4m3 expansion of ||h_j||^2 against a constant
-4 on the A side (4*e4m3(x/4) greedy residuals leave |err| <= 1/16), so the
per-column norm rides the GEMM for free.  The 3 sacrificed data dims add
zero-mean noise (std ~3.5) to each distance, far below the mining gaps.

Rows are sorted by label and each core's B columns are rotated so every
own-class (positive) column sits in the first hpj = 3 j-blocks.  Only those
blocks append a 5th e5m2 DoubleRow matmul with the -BIG one-hot mask.

The j loop walks PAIRS of 512-column blocks per row chunk so each PSUM tile
spans two banks ([128, 1024] f32); one DVE Max8 covers the pair, halving the
DVE instruction count.  Hardest-positive minima read the masked halves
directly.  Per-row losses leave the device as a [128, m_chunks] tile
(partition p, row chunk m) in one contiguous DMA; the host transposes and
does the masked mean.
"""

import functools

import numpy as np
import ml_dtypes

import concourse.bacc as bacc
import concourse.tile as tile
from concourse import mybir
from concourse.bass_utils import run_bass_kernel_spmd

FP8E4 = mybir.dt.float8e4
FP8E5 = mybir.dt.float8e5
F32 = mybir.dt.float32
BF16 = mybir.dt.bfloat16
E4 = ml_dtypes.float8_e4m3
E5 = ml_dtypes.float8_e5m2

N, D, C = 8192, 1024, 128
NCORES = 8
P = 128
JB = 512  # matmul moving free dim = one fp32 PSUM bank
HPJ = 3   # j-blocks that can contain positive (own-class) columns
ALPHA = 0.1
EPS = 1e-7
BIG = 8192.0
NNORM = 3   # e4m3 norm-expansion slots stolen from the contraction
NSCALE = 8.0  # A-side constant: each slot contributes -8 * e4m3(r/8)
              # (this e4m3 flavor saturates at 240, so r0/8 <= ~165 fits)
MODE = "fp8"


def build_program(rows, n, d, c, jb, mode=MODE, psum_bufs=4, b_bufs=5):
    """Emit the per-core Bass/Tile program (identical on all cores)."""
    kh = d // P
    m_chunks = rows // P
    nj = n // jb
    hpj = min(nj, HPJ)
    assert rows % P == 0 and d % P == 0 and n % jb == 0 and c <= P
    assert kh % 2 == 0 and nj % 2 == 0

    nc = bacc.Bacc("TRN2", target_bir_lowering=False)
    A4 = nc.dram_tensor("A4", [P, kh, rows], FP8E4, kind="ExternalInput")
    B4 = nc.dram_tensor("B4", [P, kh, n], FP8E4, kind="ExternalInput")
    A5 = nc.dram_tensor("A5", [P, 2, rows], FP8E5, kind="ExternalInput")
    B5 = nc.dram_tensor("B5", [P, 2, hpj * jb], FP8E5, kind="ExternalInput")
    loss = nc.dram_tensor("loss", [P, m_chunks], F32, kind="ExternalOutput")

    with tile.TileContext(nc) as tc:
        with (
            tc.tile_pool(name="apool", bufs=1) as apool,
            tc.tile_pool(name="bpool", bufs=b_bufs) as bpool,
            tc.tile_pool(name="psum", bufs=psum_bufs, space="PSUM") as pp,
            tc.tile_pool(name="mpool", bufs=1) as mpool,
            tc.tile_pool(name="fpool", bufs=6) as fpool,
        ):
            # Warm the PE HAM clock gate while the first DMAs land: dummy
            # matmuls on a zeroed tile keep the PE busy through its
            # 4096-cycle activity window so real matmuls run at 2.4 GHz.
            wsrc = apool.tile([1, 16 + jb], BF16, tag="wsrc")
            nc.vector.memset(wsrc[:], 0.0)
            wps = pp.tile([P, 2 * jb], F32, name="ps", tag="ps")
            for _ in range(8):
                nc.tensor.matmul(wps[:16, :jb], wsrc[:1, :16], wsrc[:1, 16:],
                                 start=True, stop=True)

            def load_b(j):
                js = slice(j * jb, (j + 1) * jb)
                b4 = bpool.tile([P, kh, jb], FP8E4, tag="b4", name="b4")
                nc.sync.dma_start(out=b4[:], in_=B4[:, :, js])
                if j >= hpj:
                    return (b4, None)
                b5 = bpool.tile([P, 2, jb], FP8E5, tag="b5", name="b5")
                nc.sync.dma_start(out=b5[:], in_=B5[:, :, js])
                return (b4, b5)

            # First moving pair before the stationary A so compute can
            # start as soon as possible.
            bpair = [None] * (nj // 2)
            bpair[0] = (load_b(0), load_b(1))

            # Stationary A, resident all kernel, loaded as one DMA per row
            # chunk so chunk m's operands land just before the PE needs
            # them (one big transfer stalled the pipeline and re-tripped
            # the HAM throttle).
            a4m = []
            a5 = None
            for m in range(m_chunks):
                ms = slice(m * P, (m + 1) * P)
                t = apool.tile([P, kh, P], FP8E4, tag=f"a4m{m}",
                               name=f"a4m{m}")
                nc.sync.dma_start(out=t[:], in_=A4[:, :, ms])
                a4m.append(t)
                if m == 0:
                    a5 = apool.tile([P, 2, rows], FP8E5, tag="a5", name="a5")
                    nc.sync.dma_start(out=a5[:], in_=A5[:])
                if m == 2 and nj > 3:
                    # Prefetch the second moving pair ahead of the
                    # remaining stationary chunks so jp=1 never stalls.
                    bpair[1] = (load_b(2), load_b(3))

            # Per-row-chunk partial mining results, merged after the j loop.
            # Max8 runs once per PSUM pair of j-blocks -> nj/2 slots of 8.
            v8 = [mpool.tile([P, nj * 4], F32, tag=f"v8_{m}", name=f"v8_{m}")
                  for m in range(m_chunks)]
            gmin = [mpool.tile([P, hpj], F32, tag=f"gm_{m}", name=f"gmin_{m}")
                    for m in range(m_chunks)]

            stage8 = mpool.tile([P, m_chunks], F32, tag="stage8")

            for jp in range(nj // 2):
                if bpair[jp] is None:
                    bpair[jp] = (load_b(2 * jp), load_b(2 * jp + 1))

                for m in range(m_chunks):
                    ps = pp.tile([P, 2 * jb], F32, name="ps", tag="ps")
                    at = a4m[m]
                    for half in range(2):
                        j = 2 * jp + half
                        b4, b5 = bpair[jp][half]
                        ph = ps[:, half * jb:(half + 1) * jb]
                        for t in range(kh // 2):
                            nc.tensor.matmul(
                                ph, at[:, 2 * t:2 * t + 2, :],
                                b4[:, 2 * t:2 * t + 2, :],
                                start=(t == 0),
                                stop=(t == kh // 2 - 1 and j >= hpj),
                                perf_mode=mybir.MatmulPerfMode.DoubleRow)
                        if j < hpj:
                            # Masked block: -BIG one-hot rides a 5th matmul.
                            nc.tensor.matmul(
                                ph, a5[:, :, m * P:(m + 1) * P], b5[:],
                                start=False, stop=True,
                                perf_mode=mybir.MatmulPerfMode.DoubleRow)
                            # hp mining straight off the masked half.
                            nc.vector.tensor_reduce(
                                gmin[m][:, j:j + 1], ph,
                                axis=mybir.AxisListType.X,
                                op=mybir.AluOpType.min)

                    nc.vector.max(v8[m][:, jp * 8:(jp + 1) * 8], ps[:])

                    if jp == nj // 2 - 1:
                        # Final merge for this row chunk, interleaved so it
                        # overlaps the remaining row chunks' matmuls.
                        vf = fpool.tile([P, 8], F32, tag="vf", name="vf")
                        nc.vector.max(vf[:], v8[m][:])
                        gm = fpool.tile([P, 1], F32, tag="gm", name="gm")
                        nc.vector.tensor_reduce(gm[:], gmin[m][:],
                                                axis=mybir.AxisListType.X,
                                                op=mybir.AluOpType.min)
                        # loss_pre = (v2 + (ALPHA - BIG)) - gmin
                        nc.vector.scalar_tensor_tensor(
                            out=stage8[:, m:m + 1], in0=vf[:, 1:2],
                            scalar=float(ALPHA - BIG), in1=gm[:],
                            op0=mybir.AluOpType.add,
                            op1=mybir.AluOpType.subtract)

            # One contiguous [P, m_chunks] DMA; the host transposes.
            nc.sync.dma_start(out=loss[:], in_=stage8[:])

    nc.compile()
    return nc


def _split_e4(x, terms, scale):
    """Greedy expansion: x ~ scale * sum of `terms` e4m3 rows (f64 in/out)."""
    out = []
    r = x.astype(np.float64).copy()
    for _ in range(terms):
        s = (r / scale).astype(E4)
        out.append(s)
        r -= scale * s.astype(np.float64)
    return out


def make_inputs(H, labels, n, d, c, ncores, mode=MODE):
    """Host-side packing of the augmented GEMM operands.

    Rows are sorted by label and sharded contiguously.  Each core's B
    columns are rotated so every column whose label appears among that
    core's rows sits in the leading block (always < HPJ * JB columns), which
    lets the device mine the hardest positive from the first HPJ j-blocks
    only and skip the mask matmul everywhere else.  The final masked mean
    is permutation invariant, so neither the sort nor the rotations need
    undoing.

    The last NNORM contraction slots of A4/B4 are repurposed: A-side holds
    the constant -NSCALE, B-side the greedy e4m3 expansion of ||h_j||^2
    (computed over the SURVIVING d - NNORM dims' quantized values plus the
    full-precision tail, see below), so p_ij needs no extra matmul.
    """
    H = np.ascontiguousarray(np.asarray(H, dtype=np.float32))
    labels = np.asarray(labels).astype(np.int64).ravel()
    kh = d // P
    rows = n // ncores
    nj = n // JB
    hpj = min(nj, HPJ)

    perm = np.argsort(labels, kind="stable")
    H = H[perm]
    labels = labels[perm]
    col_orders = []
    for cix in range(ncores):
        own = np.zeros(n, dtype=bool)
        own[np.isin(labels, labels[cix * rows:(cix + 1) * rows])] = True
        order = np.concatenate([np.nonzero(own)[0], np.nonzero(~own)[0]])
        assert own.sum() <= min(n, hpj * JB), own.sum()
        col_orders.append(order)

    oh = labels[None, :] == np.arange(c, dtype=np.int64)[:, None]  # [c, n]

    Hr = H.astype(E4)
    # Full-data norm (all d dims, at e4m3 precision) -- matches the
    # reference's ||h||^2 term; the dot product just loses the last NNORM
    # dims (zero-mean noise on each distance).
    xn = np.einsum("ij,ij->i", Hr.astype(np.float64), Hr.astype(np.float64))
    xsplit = _split_e4(xn, NNORM, NSCALE)

    B4m = Hr.T.reshape(kh, P, n).transpose(1, 0, 2).copy()  # [P, kh, n] e4m3
    for t in range(NNORM):
        B4m[P - NNORM + t, kh - 1, :] = xsplit[t]
    B5m = np.zeros((P, 2, n), dtype=E5)
    B5m[:c, 0, :] = oh.astype(E5)

    in_maps = []
    for cix in range(ncores):
        sl = slice(cix * rows, (cix + 1) * rows)
        order = col_orders[cix]
        A4m = ((2.0 * Hr.astype(np.float32)[sl].T).astype(E4)
               .reshape(kh, P, rows).transpose(1, 0, 2).copy())
        A4m[P - NNORM:, kh - 1, :] = -NSCALE
        A5m = np.zeros((P, 2, rows), dtype=E5)
        A5m[:c, 0, :] = (-BIG * oh[:, sl]).astype(E5)
        in_maps.append({"A4": A4m, "B4": B4m[:, :, order],
                        "A5": A5m, "B5": B5m[:, :, order[:hpj * JB]]})
    return in_maps


@functools.lru_cache(maxsize=2)
def _get_program(mode=MODE):
    return build_program(N // NCORES, N, D, C, JB, mode=mode)


def _finalize(loss_rows):
    loss_all = np.concatenate(
        [np.asarray(l, dtype=np.float64).T.ravel() for l in loss_rows])
    loss_all = np.maximum(loss_all, 0.0)
    rel = loss_all > EPS
    cnt = int(rel.sum())
    if cnt == 0:
        return np.float32(0.0)
    return np.float32(loss_all[rel].sum() / cnt)


def kernel(H, labels):
    in_maps = make_inputs(H, labels, N, D, C, NCORES)
    res = run_bass_kernel_spmd(_get_program(), in_maps, list(range(NCORES)))
    return _finalize([r["loss"] for r in res.results])
